# revision 29
# baseline (speedup 1.0000x reference)
"""TRN2 Bass kernel for nn_Attention_17935783428543.

Reference computation (per batch b of 4):
  qkv = w_qkv @ X        (X = x[b] as [C=128, N=4096])
  per head h (4 heads, d=32): sim = (q_h * scale)^T k_h ; P = softmax(sim)
  y_h = P @ v_h^T ; out = w_out @ concat_h(y_h^T) + b_out

Sharding: 8 cores = 4 batches x 2 query-halves. Each core computes the full
attention for its batch restricted to 2048 query pixels (all 4096 keys), all
4 heads, including QKV projection and the output projection. No collectives.
The query-half assignment uses a host-side rotation of x's pixel axis so all
8 cores run the identical SPMD graph: queries are always columns 0:2048.

Per-core design (all matmuls bf16, PSUM f32). The kernel's wall is the
softmax exp: 33.5M elements/core, and ACT alone runs 1 elem/lane/cycle
@1.2GHz (~262us with overheads). Two levers vs the previous version:

  1. exp is SPLIT between ACT and DVE. DVE windows compute exp with a
     Schraudolph bit-trick in ONE tensor_scalar: i16 = int(sim*a + b) where
     a = 2^7/ln2, b = 127*2^7 - c. The int16 bit pattern IS the bf16 of
     exp(sim) (rel err ~1.8% rms, softmax cancels the constant factor;
     verified end-to-end rel err 6.5e-3 vs the 2e-2 gate). Window pattern
     A,D,A,D,A balances ACT (1573ns/1536-window) vs DVE (1795ns + its copy
     and epilogue work).
  2. softmax denominators are FREE: the P@V matmuls use lhsT = [v_h | ones32]
     (M=64; matmul cost depends only on N) so each PV matmul accumulates
     y_h^T in rows 0:32 and the broadcast row-sums in rows 32:64 of its
     64-row block. The previous pair-sum DVE ops + ones-matmul stream are
     gone entirely.

Layouts: sim^T tiles [j=128, i=512] per head via K=32 row-packed
tile_position (heads on PE row groups 32h); three head-sims per [128,1536]
PSUM window (3 banks), ring double-buffered (2x3 banks). PV col-packs heads
pairwise: acc tile t = h>>1 holds [y_h|sums_h, y_h'|sums_h'] (2 banks).
Epilogue per i-chunk: whole-tile fast reciprocal, 4x [32,512] ynorm mults,
w_out^T projection (+bias), DMA out; outproj PSUM borrows an acc bank.
QKV projections stream through the sim PSUM ring just-in-time so they never
pile PE/DVE work into the ramp windows.
"""

import math

import numpy as np
import ml_dtypes

import concourse.mybir as mybir
import concourse.tile as tile
from concourse import bacc
from concourse.bass_utils import run_bass_kernel_spmd

F32 = mybir.dt.float32
BF16 = mybir.dt.bfloat16
I16 = mybir.dt.int16
NPBF16 = ml_dtypes.bfloat16

B = 4
C = 128
HEADS = 4
D = 32
N = 4096          # pixels per batch (64*64)
NQ = 2048         # query pixels per core
SCALE = D ** -0.5
I_CHUNK = 512
J_CHUNK = 128
N_I = NQ // I_CHUNK     # 4
N_J = N // J_CHUNK      # 32

# Schraudolph exp in bf16-bit domain (sims arrive pre-scaled via wq):
#   bf16_bits(exp(s)) ~= int16(s * 2^7/ln2 + (127*2^7 - c))
# c=3.25 splits the optimum between round-to-nearest (3.5) and truncating
# (3.0) f32->i16 conversion; softmax cancels the resulting constant factor.
A_EXP = 128.0 / math.log(2.0)
B_EXP = 16256.0 - 3.25

# per-mille of windows on ACT (rest on DVE Schraudolph); balanced so both
# engines' total busy (exp + copies vs exp + epilogues) come out equal
ACT_FRAC = 520

_NC_CACHE = {}


def _build_nc():
    nc = bacc.Bacc("TRN2", target_bir_lowering=False, debug=False, num_devices=8)

    x = nc.dram_tensor("x", [N // 512, C, 512], BF16, kind="ExternalInput").ap()
    wq = nc.dram_tensor("wq", [C, C], BF16, kind="ExternalInput").ap()
    wk = nc.dram_tensor("wk", [C, C], BF16, kind="ExternalInput").ap()
    wv = nc.dram_tensor("wv", [C, C], BF16, kind="ExternalInput").ap()
    wo = nc.dram_tensor("wo", [C, C], BF16, kind="ExternalInput").ap()
    out = nc.dram_tensor("out", [C, NQ], F32, kind="ExternalOutput").ap()

    with tile.TileContext(nc) as tc:
        with (
            tc.tile_pool(name="const", bufs=1) as cpool,
            tc.tile_pool(name="acts", bufs=1) as apool,
            tc.tile_pool(name="pt", bufs=8) as ptpool,
            tc.tile_pool(name="epi", bufs=2) as epool,
            tc.tile_pool(name="psum_ring", bufs=2, space="PSUM") as pring,
            tc.tile_pool(name="psum_acc", bufs=1, space="PSUM") as pacc,
        ):
            # ---- constants / weights ----
            wq_sb = cpool.tile([C, C], BF16, tag="wq")
            nc.sync.dma_start(wq_sb[:], wq)
            wk_sb = cpool.tile([C, C], BF16, tag="wk")
            nc.sync.dma_start(wk_sb[:], wk)
            wv_sb = cpool.tile([C, C], BF16, tag="wv")
            nc.sync.dma_start(wv_sb[:], wv)
            wo_sb = cpool.tile([C, C], BF16, tag="wo")
            nc.sync.dma_start(wo_sb[:], wo)

            # warm the ACT exp table during the DMA prologue
            warm = cpool.tile([1, 1], F32, tag="warm")
            nc.vector.memset(warm[:], 0.0)
            nc.scalar.activation(warm[:], warm[:], mybir.ActivationFunctionType.Exp)

            # ---- x DMA (8 chunks so early projections can start early) ----
            x_sb = apool.tile([C, N], BF16, tag="x")
            for g in range(N // 512):
                nc.gpsimd.dma_start(x_sb[:, 512 * g : 512 * (g + 1)], x[g])

            q_all = apool.tile([C, NQ], BF16, tag="q")    # [4h*32c', i]
            k_all = apool.tile([C, N], BF16, tag="k")     # [4h*32c', j]
            # v^T with interleaved ones columns: per chunk J the 256-col block
            # is [v_h0(32) | 1(32) | v_h1 | 1 | v_h2 | 1 | v_h3 | 1], rows = j
            vT_all = apool.tile([C, 256 * N_J], BF16, tag="vT")

            def emit_ones_memset():
                # fill only the 128 ones-slots (odd 32-col slots), strided
                ones_ap = vT_all[:].rearrange("p (a d) -> p a d", d=64)[:, :, 32:64]
                nc.vector.memset(ones_ap, 1.0)

            # ---- projection units ----
            # A unit occupies one 512-col region of a PSUM tile (one bank):
            # either a standalone prologue ring tile or a window slot.
            def emit_proj_unit(kind, g, ps, sl):
                if kind == "q":
                    nc.tensor.matmul(
                        ps[:, sl],
                        lhsT=wq_sb[:],
                        rhs=x_sb[:, 512 * g : 512 * (g + 1)],
                        start=True,
                        stop=True,
                    )
                    nc.vector.tensor_copy(q_all[:, 512 * g : 512 * (g + 1)], ps[:, sl])
                elif kind == "k":
                    nc.tensor.matmul(
                        ps[:, sl],
                        lhsT=wk_sb[:],
                        rhs=x_sb[:, 512 * g : 512 * (g + 1)],
                        start=True,
                        stop=True,
                    )
                    nc.scalar.activation(
                        k_all[:, 512 * g : 512 * (g + 1)], ps[:, sl],
                        mybir.ActivationFunctionType.Copy,
                    )
                else:  # v^T: per 128-pixel chunk, out[j, o'] = x^T wv
                    for c4 in range(4):
                        J = 4 * g + c4
                        nc.tensor.matmul(
                            ps[:, sl.start + 128 * c4 : sl.start + 128 * (c4 + 1)],
                            lhsT=x_sb[:, 128 * J : 128 * (J + 1)],
                            rhs=wv_sb[:],
                            start=True,
                            stop=True,
                        )
                    # scatter the 16 [?,32] head-slices into the v-slots
                    # (even 32-col slots of the 4 chunks' 256-col blocks)
                    dst = (
                        vT_all[:, 1024 * g : 1024 * (g + 1)]
                        .rearrange("p (a d) -> p a d", d=64)[:, :, 0:32]
                    )
                    src = ps[:, sl].rearrange("p (a d) -> p a d", d=32)
                    nc.scalar.activation(dst, src, mybir.ActivationFunctionType.Copy)

            def emit_proj_batch(units):
                width = 512 * len(units)
                ps = pring.tile([128, width], F32, tag="ring", name="proj_ps")
                for u, (kind, g) in enumerate(units):
                    emit_proj_unit(kind, g, ps, slice(512 * u, 512 * (u + 1)))

            # ---- attention stream ----
            # deferral scheduler: actions run at the start of their due body,
            # in insertion order (bodies past the window list keep flushing)
            acc_tiles = {}
            pending = []  # (due_body, fn)

            def schedule(due, fn):
                pending.append((due, fn))

            def flush(w):
                i = 0
                while i < len(pending):
                    due, fn = pending[i]
                    if due <= w:
                        pending.pop(i)
                        fn()
                    else:
                        i += 1

            pv_queue = []

            def emit_pv_slot(I, J, h, pTw, k, w):
                if I not in acc_tiles:
                    acc_tiles[I] = (
                        pacc.tile([128, I_CHUNK], F32, tag="acc0", name="acc0"),
                        pacc.tile([128, I_CHUNK], F32, tag="acc1", name="acc1"),
                    )
                base = 64 * (h & 1)
                nc.tensor.matmul(
                    acc_tiles[I][h >> 1][base : base + 64, :],
                    lhsT=vT_all[:, 256 * J + 64 * h : 256 * J + 64 * h + 64],
                    rhs=pTw[:, 512 * k : 512 * (k + 1)],
                    start=(J == 0),
                    stop=(J == N_J - 1),
                    tile_position=(0, base),
                )
                if (J, h) == (N_J - 1, HEADS - 1):
                    st = emit_epilogue1a(I)
                    schedule(w + 1, lambda st=st: emit_epilogue1b(st))
                    schedule(w + 2, lambda I=I, st=st: emit_epilogue2(I, st))

            def emit_pv(slots, pTw, w, flush_all):
                # drain PVs in PAIRS: consecutive head slots alternate col
                # parity, so each pair runs on disjoint PE col-groups with no
                # serialization (a 3-PV group always repeats one col-group)
                for k, (_, I, J, h) in enumerate(slots):
                    pv_queue.append((I, J, h, pTw, k))
                while len(pv_queue) >= 2:
                    for _ in range(2):
                        emit_pv_slot(*pv_queue.pop(0), w)
                if flush_all and pv_queue:
                    emit_pv_slot(*pv_queue.pop(0), w)

            def emit_epilogue1a(I):
                acc0, acc1 = acc_tiles.pop(I)
                r0 = epool.tile([128, I_CHUNK], F32, tag="r0")
                nc.vector.reciprocal_approx_fast(r0[:], acc0[:])
                r1 = epool.tile([128, I_CHUNK], F32, tag="r1")
                nc.vector.reciprocal_approx_fast(r1[:], acc1[:])
                ynorm = epool.tile([128, I_CHUNK], BF16, tag="ynorm")
                for h in (0, 1):
                    nc.vector.tensor_tensor(
                        ynorm[32 * h : 32 * h + 32, :],
                        acc0[64 * h : 64 * h + 32, :],
                        r0[64 * h + 32 : 64 * h + 64, :],
                        mybir.AluOpType.mult,
                    )
                return (acc1, r1, ynorm)

            def emit_epilogue1b(st):
                acc1, r1, ynorm = st
                for h in (2, 3):
                    b = 64 * (h & 1)
                    nc.vector.tensor_tensor(
                        ynorm[32 * h : 32 * h + 32, :],
                        acc1[b : b + 32, :],
                        r1[b + 32 : b + 64, :],
                        mybir.AluOpType.mult,
                    )

            def emit_epilogue2(I, st):
                ynorm = st[2]
                isl = slice(I_CHUNK * I, I_CHUNK * (I + 1))
                # outproj PSUM rides the ring (one window slot per i-chunk):
                # borrowing an acc bank makes the pool's generation order
                # chain it behind the NEXT chunk's accumulation
                op = pring.tile([128, I_CHUNK], F32, tag="ring", name="op")
                nc.tensor.matmul(op[:], lhsT=wo_sb[:], rhs=ynorm[:], start=True, stop=True)
                out_sb = epool.tile([128, I_CHUNK], F32, tag="out_sb")
                nc.vector.tensor_copy(out_sb[:], op[:])
                nc.sync.dma_start(out[:, isl], out_sb[:])

            def emit_window(slots, w, nact, last):
                # slots: sim slots first, then proj slots (exp reads a
                # contiguous sim prefix; proj regions feed their ACT copy)
                sims = [s for s in slots if s[0] == "sim"]
                width = 512 * len(slots)
                simw = pring.tile([128, width], F32, tag="ring", name="simw")
                for k, (_, I, J, h) in enumerate(sims):
                    nc.tensor.matmul(
                        simw[:, 512 * k : 512 * (k + 1)],
                        lhsT=k_all[32 * h : 32 * h + 32, 128 * J : 128 * (J + 1)],
                        rhs=q_all[32 * h : 32 * h + 32, I_CHUNK * I : I_CHUNK * (I + 1)],
                        start=True,
                        stop=True,
                        tile_position=(32 * h, 0),
                    )
                # proj writes BEFORE the exp read: tile-granularity dep
                # tracking must never chain a PE write behind the exp
                for k, (_, kind, g) in enumerate(slots[len(sims) :]):
                    sl = slice(512 * (len(sims) + k), 512 * (len(sims) + k + 1))
                    emit_proj_unit(kind, g, simw, sl)
                ew = 512 * len(sims)
                pTw = ptpool.tile([128, ew], BF16, tag="pT", name="pTw")
                if nact:
                    nc.scalar.activation(
                        pTw[:], simw[:, 0:ew], mybir.ActivationFunctionType.Exp
                    )
                else:
                    nc.vector.tensor_scalar(
                        pTw[:].bitcast(I16),
                        simw[:, 0:ew],
                        A_EXP,
                        B_EXP,
                        mybir.AluOpType.mult,
                        mybir.AluOpType.add,
                    )
                flush(w)
                # PV defers TWO bodies so the exp latency chain (sims -> sem
                # -> exp -> sem -> PV) never stalls the PE: micro-idles would
                # re-throttle the HAM clock gate to 1.2GHz and double every
                # matmul's duration. (+1 at the stream tail to shorten drain.)
                d = 1 if last else 3
                schedule(w + d, lambda: emit_pv(sims, pTw, w + d, last))

            # slot stream: per i-chunk 128 head-sims; proj units ride in
            # i-chunk 0's stream (one per 8 sims — each unit's ~0.5-1us PE
            # burst must fit the per-window slack or it trips a stall that
            # HAM-cools the PE), so they never steal a PSUM ring slot.
            # v0/k1 go into the first windows; k0/q0 prime via the prologue.
            proj_units = [
                ("v", 1), ("k", 2), ("v", 2), ("k", 3), ("v", 3), ("k", 4),
                ("v", 4), ("k", 5), ("v", 5), ("k", 6), ("v", 6), ("k", 7),
                ("v", 7), ("q", 1), ("q", 2), ("q", 3),
            ]
            stream = []
            uq = list(proj_units)
            for I in range(N_I):
                for t in range(N_J * HEADS):
                    stream.append(("sim", I, t // 4, t % 4))
                    if I == 0 and t == 1:
                        stream.append(("proj", "v", 0))
                    elif I == 0 and t == 3:
                        stream.append(("proj", "k", 1))
                    elif I == 0 and uq and t % 8 == 7:
                        stream.append(("proj",) + uq.pop(0))
            assert not uq

            # pack 3 slots per window, sims first within a window; force ACT
            # windows right after each i-chunk boundary (the DVE is busy with
            # the previous chunk's epilogue there)
            windows = []
            for w0 in range(0, len(stream), 3):
                slots = stream[w0 : w0 + 3]
                slots.sort(key=lambda s: s[0] != "sim")
                windows.append(slots)

            emit_proj_batch([("k", 0), ("q", 0)])
            for w, slots in enumerate(windows):
                if w == 1:
                    emit_ones_memset()
                sims = [s for s in slots if s[0] == "sim"]
                # Bresenham engine assignment at ACT_FRAC (balances ACT busy
                # incl. proj copies vs DVE busy incl. epilogues)
                nact = (w * ACT_FRAC) // 1000 != ((w - 1) * ACT_FRAC) // 1000
                if len(sims) < len(slots):
                    nact = True  # proj windows: shortest exp, ACT's copy
                if sims and sims[0][2] <= 2 and sims[0][1] > 0:
                    nact = True  # chunk-boundary: epilogue owns the DVE
                emit_window(slots, w, nact, last=(w >= len(windows) - 2))
            w = len(windows)
            while pending:
                flush(w)
                w += 1

    nc.compile()
    return nc


def kernel(x, w_qkv, w_out, b_out, _trace=False):
    if "nc" not in _NC_CACHE:
        _NC_CACHE["nc"] = _build_nc()
    nc = _NC_CACHE["nc"]

    x = np.asarray(x, dtype=np.float32).reshape(B, C, N)
    w_qkv = np.asarray(w_qkv, dtype=np.float32)
    w_out = np.asarray(w_out, dtype=np.float32)
    b_out = np.asarray(b_out, dtype=np.float32)

    # SCALE folded into wq so sims arrive pre-scaled (both exp paths use raw)
    wq = np.ascontiguousarray(w_qkv[0:C].T * SCALE).astype(NPBF16)
    wk = np.ascontiguousarray(w_qkv[C : 2 * C].T).astype(NPBF16)
    wv = np.ascontiguousarray(w_qkv[2 * C : 3 * C].T).astype(NPBF16)
    wo = np.ascontiguousarray(w_out.T).astype(NPBF16)

    in_maps = []
    for core in range(8):
        b, half = core >> 1, core & 1
        xb = x[b]
        if half:
            xb = np.concatenate([xb[:, NQ:], xb[:, :NQ]], axis=1)
        xb_c = np.ascontiguousarray(
            xb.reshape(C, N // 512, 512).transpose(1, 0, 2)
        ).astype(NPBF16)
        in_maps.append(
            {
                "x": xb_c,
                "wq": wq,
                "wk": wk,
                "wv": wv,
                "wo": wo,
            }
        )

    res = run_bass_kernel_spmd(nc, in_maps, list(range(8)), trace=_trace)

    full = np.empty((B, C, N), np.float32)
    for core in range(8):
        b, half = core >> 1, core & 1
        full[b][:, NQ * half : NQ * (half + 1)] = res.results[core]["out"]
    full += b_out[None, :, None]
    out = full.reshape(B, C, 64, 64)
    if _trace:
        return out, res
    return out


# revision 34
# speedup vs baseline: 1.0302x; 1.0302x over previous
"""TRN2 Bass kernel for nn_Attention_17935783428543.

Reference computation (per batch b of 4):
  qkv = w_qkv @ X        (X = x[b] as [C=128, N=4096])
  per head h (4 heads, d=32): sim = (q_h * scale)^T k_h ; P = softmax(sim)
  y_h = P @ v_h^T ; out = w_out @ concat_h(y_h^T) + b_out

Sharding: 8 cores = 4 batches x 2 query-halves. Each core computes the full
attention for its batch restricted to 2048 query pixels (all 4096 keys), all
4 heads, including QKV projection and the output projection. No collectives.
The query-half assignment uses a host-side rotation of x's pixel axis so all
8 cores run the identical SPMD graph: queries are always columns 0:2048.

Per-core design (all matmuls bf16, PSUM f32). The kernel's wall is the
softmax exp: 33.5M elements/core, and ACT alone runs 1 elem/lane/cycle
@1.2GHz (~262us with overheads). Two levers vs the previous version:

  1. exp is SPLIT between ACT and DVE. DVE windows compute exp with a
     Schraudolph bit-trick in ONE tensor_scalar: i16 = int(sim*a + b) where
     a = 2^7/ln2, b = 127*2^7 - c. The int16 bit pattern IS the bf16 of
     exp(sim) (rel err ~1.8% rms, softmax cancels the constant factor;
     measured end-to-end rel err 5.9e-3 vs the 2e-2 gate). Window pattern
     A,D,A,D,A balances ACT (1573ns/1536-window) vs DVE (1795ns + its
     epilogue work); chunk-boundary windows force ACT so the DVE can run
     the epilogue.
  2. softmax denominators are FREE: the P@V matmuls use lhsT = [v_h | ones32]
     (M=64; matmul cost depends only on N) so each PV matmul accumulates
     y_h^T in rows 0:32 and the broadcast row-sums in rows 32:64 of its
     64-row block. The previous pair-sum DVE ops + ones-matmul stream are
     gone entirely.

Layouts: sim^T tiles [j=128, i=512] per head via K=32 row-packed
tile_position (heads on PE row groups 32h); three head-sims per [128,1536]
PSUM window (3 banks), ring double-buffered (2x3 banks). PV col-packs heads
pairwise: acc tile t = h>>1 holds [y_h|sums_h, y_h'|sums_h'] (2 banks).
Epilogue per i-chunk (split over three window bodies to keep DVE bursts off
the exp stream): whole-tile fast reciprocal, 4x [32,512] ynorm mults, w_out^T
projection, ACT-copy to SBUF, DMA out. The outproj PSUM rides the sim ring
(borrowing an acc bank would chain it behind the NEXT chunk's accumulation
via the pool's generation order); b_out is added on the host. QKV projection
units ride inside window slots (one per 8 sims) with ACT-side copies, so
they never steal a PSUM ring slot; their PE bursts are the main residual
stall source. PV emission is deferred two bodies: the exp latency chain
(sims -> sem -> exp -> sem -> PV) must never idle the PE, because HAM
re-throttles an idling PE to 1.2GHz and doubles every matmul duration.
Measured: ~250us (baseline 333us): ACT ~160us, DVE ~155us, PE ~185us busy.
"""

import math

import numpy as np
import ml_dtypes

import concourse.mybir as mybir
import concourse.tile as tile
from concourse import bacc
from concourse.bass_utils import run_bass_kernel_spmd

F32 = mybir.dt.float32
BF16 = mybir.dt.bfloat16
I16 = mybir.dt.int16
NPBF16 = ml_dtypes.bfloat16

B = 4
C = 128
HEADS = 4
D = 32
N = 4096          # pixels per batch (64*64)
NQ = 2048         # query pixels per core
SCALE = D ** -0.5
I_CHUNK = 512
J_CHUNK = 128
N_I = NQ // I_CHUNK     # 4
N_J = N // J_CHUNK      # 32

# Schraudolph exp in bf16-bit domain (sims arrive pre-scaled via wq):
#   bf16_bits(exp(s)) ~= int16(s * 2^7/ln2 + (127*2^7 - c))
# c=3.25 splits the optimum between round-to-nearest (3.5) and truncating
# (3.0) f32->i16 conversion; softmax cancels the resulting constant factor.
A_EXP = 128.0 / math.log(2.0)
B_EXP = 16256.0 - 3.25

# per-window exp engine: A=ACT exact exp, D=DVE Schraudolph (3:2 balance)
ENGINE_PATTERN = "ADADA"

_NC_CACHE = {}


def _build_nc():
    nc = bacc.Bacc("TRN2", target_bir_lowering=False, debug=False, num_devices=8)

    x = nc.dram_tensor("x", [N // 512, C, 512], BF16, kind="ExternalInput").ap()
    wq = nc.dram_tensor("wq", [C, C], BF16, kind="ExternalInput").ap()
    wk = nc.dram_tensor("wk", [C, C], BF16, kind="ExternalInput").ap()
    wv = nc.dram_tensor("wv", [C, C], BF16, kind="ExternalInput").ap()
    wo = nc.dram_tensor("wo", [C, C], BF16, kind="ExternalInput").ap()
    out = nc.dram_tensor("out", [C, NQ], F32, kind="ExternalOutput").ap()

    with tile.TileContext(nc) as tc:
        with (
            tc.tile_pool(name="const", bufs=1) as cpool,
            tc.tile_pool(name="acts", bufs=1) as apool,
            tc.tile_pool(name="pt", bufs=6) as ptpool,
            tc.tile_pool(name="epi", bufs=2) as epool,
            tc.tile_pool(name="psum_ring", bufs=2, space="PSUM") as pring,
            tc.tile_pool(name="psum_acc", bufs=1, space="PSUM") as pacc,
        ):
            # ---- constants / weights ----
            wq_sb = cpool.tile([C, C], BF16, tag="wq")
            nc.sync.dma_start(wq_sb[:], wq)
            wk_sb = cpool.tile([C, C], BF16, tag="wk")
            nc.sync.dma_start(wk_sb[:], wk)
            wv_sb = cpool.tile([C, C], BF16, tag="wv")
            nc.sync.dma_start(wv_sb[:], wv)
            wo_sb = cpool.tile([C, C], BF16, tag="wo")
            nc.sync.dma_start(wo_sb[:], wo)

            # warm the ACT exp table during the DMA prologue
            warm = cpool.tile([1, 1], F32, tag="warm")
            nc.vector.memset(warm[:], 0.0)
            nc.scalar.activation(warm[:], warm[:], mybir.ActivationFunctionType.Exp)

            # ---- x DMA (8 chunks so early projections can start early) ----
            x_sb = apool.tile([C, N], BF16, tag="x")
            for g in range(N // 512):
                nc.gpsimd.dma_start(x_sb[:, 512 * g : 512 * (g + 1)], x[g])

            q_all = apool.tile([C, NQ], BF16, tag="q")    # [4h*32c', i]
            k_all = apool.tile([C, N], BF16, tag="k")     # [4h*32c', j]
            # v^T with interleaved ones columns: per chunk J the 256-col block
            # is [v_h0(32) | 1(32) | v_h1 | 1 | v_h2 | 1 | v_h3 | 1], rows = j
            vT_all = apool.tile([C, 256 * N_J], BF16, tag="vT")

            def emit_ones_memset():
                # fill only the 128 ones-slots (odd 32-col slots), strided
                ones_ap = vT_all[:].rearrange("p (a d) -> p a d", d=64)[:, :, 32:64]
                nc.vector.memset(ones_ap, 1.0)

            # ---- projection units ----
            # A unit occupies one 512-col region of a PSUM tile (one bank):
            # either a standalone prologue ring tile or a window slot.
            def emit_proj_unit(kind, g, ps, sl, dve_copy=False):
                if kind == "q" and dve_copy:
                    nc.tensor.matmul(
                        ps[:, sl],
                        lhsT=wq_sb[:],
                        rhs=x_sb[:, 512 * g : 512 * (g + 1)],
                        start=True,
                        stop=True,
                    )
                    nc.vector.tensor_copy(q_all[:, 512 * g : 512 * (g + 1)], ps[:, sl])
                elif kind == "q":
                    nc.tensor.matmul(
                        ps[:, sl],
                        lhsT=wq_sb[:],
                        rhs=x_sb[:, 512 * g : 512 * (g + 1)],
                        start=True,
                        stop=True,
                    )
                    nc.scalar.activation(
                        q_all[:, 512 * g : 512 * (g + 1)], ps[:, sl],
                        mybir.ActivationFunctionType.Copy,
                    )
                elif kind == "k":
                    nc.tensor.matmul(
                        ps[:, sl],
                        lhsT=wk_sb[:],
                        rhs=x_sb[:, 512 * g : 512 * (g + 1)],
                        start=True,
                        stop=True,
                    )
                    nc.scalar.activation(
                        k_all[:, 512 * g : 512 * (g + 1)], ps[:, sl],
                        mybir.ActivationFunctionType.Copy,
                    )
                else:  # v^T: per 128-pixel chunk, out[j, o'] = x^T wv
                    for c4 in range(4):
                        J = 4 * g + c4
                        nc.tensor.matmul(
                            ps[:, sl.start + 128 * c4 : sl.start + 128 * (c4 + 1)],
                            lhsT=x_sb[:, 128 * J : 128 * (J + 1)],
                            rhs=wv_sb[:],
                            start=True,
                            stop=True,
                        )
                    # scatter the 16 [?,32] head-slices into the v-slots
                    # (even 32-col slots of the 4 chunks' 256-col blocks)
                    dst = (
                        vT_all[:, 1024 * g : 1024 * (g + 1)]
                        .rearrange("p (a d) -> p a d", d=64)[:, :, 0:32]
                    )
                    src = ps[:, sl].rearrange("p (a d) -> p a d", d=32)
                    nc.scalar.activation(dst, src, mybir.ActivationFunctionType.Copy)

            def emit_proj_batch(units):
                width = 512 * len(units)
                ps = pring.tile([128, width], F32, tag="ring", name="proj_ps")
                for u, (kind, g) in enumerate(units):
                    emit_proj_unit(
                        kind, g, ps, slice(512 * u, 512 * (u + 1)), dve_copy=True
                    )

            # ---- attention stream ----
            # deferral scheduler: actions run at the start of their due body,
            # in insertion order (bodies past the window list keep flushing)
            acc_tiles = {}
            pending = []  # (due_body, fn)

            def schedule(due, fn):
                pending.append((due, fn))

            def flush(w):
                i = 0
                while i < len(pending):
                    due, fn = pending[i]
                    if due <= w:
                        pending.pop(i)
                        fn()
                    else:
                        i += 1

            def emit_pv_slot(I, J, h, pTw, k, w):
                if I not in acc_tiles:
                    acc_tiles[I] = (
                        pacc.tile([128, I_CHUNK], F32, tag="acc0", name="acc0"),
                        pacc.tile([128, I_CHUNK], F32, tag="acc1", name="acc1"),
                    )
                base = 64 * (h & 1)
                nc.tensor.matmul(
                    acc_tiles[I][h >> 1][base : base + 64, :],
                    lhsT=vT_all[:, 256 * J + 64 * h : 256 * J + 64 * h + 64],
                    rhs=pTw[:, 512 * k : 512 * (k + 1)],
                    start=(J == 0),
                    stop=(J == N_J - 1),
                    tile_position=(0, base),
                )
                if (J, h) == (N_J - 1, HEADS - 1):
                    st = emit_epilogue1a(I)
                    schedule(w + 1, lambda st=st: emit_epilogue1b(st))
                    schedule(w + 2, lambda I=I, st=st: emit_epilogue2(I, st))

            def emit_pv(slots, pTw, w):
                for k, (_, I, J, h) in enumerate(slots):
                    emit_pv_slot(I, J, h, pTw, k, w)

            def emit_epilogue1a(I):
                acc0, acc1 = acc_tiles.pop(I)
                r0 = epool.tile([128, I_CHUNK], F32, tag="r0")
                nc.vector.reciprocal_approx_fast(r0[:], acc0[:])
                r1 = epool.tile([128, I_CHUNK], F32, tag="r1")
                nc.vector.reciprocal_approx_fast(r1[:], acc1[:])
                ynorm = epool.tile([128, I_CHUNK], BF16, tag="ynorm")
                for h in (0, 1):
                    nc.vector.tensor_tensor(
                        ynorm[32 * h : 32 * h + 32, :],
                        acc0[64 * h : 64 * h + 32, :],
                        r0[64 * h + 32 : 64 * h + 64, :],
                        mybir.AluOpType.mult,
                    )
                return (acc1, r1, ynorm)

            def emit_epilogue1b(st):
                acc1, r1, ynorm = st
                for h in (2, 3):
                    b = 64 * (h & 1)
                    nc.vector.tensor_tensor(
                        ynorm[32 * h : 32 * h + 32, :],
                        acc1[b : b + 32, :],
                        r1[b + 32 : b + 64, :],
                        mybir.AluOpType.mult,
                    )

            def emit_epilogue2(I, st):
                ynorm = st[2]
                isl = slice(I_CHUNK * I, I_CHUNK * (I + 1))
                # outproj PSUM rides the ring (one window slot per i-chunk):
                # borrowing an acc bank makes the pool's generation order
                # chain it behind the NEXT chunk's accumulation
                op = pring.tile([128, I_CHUNK], F32, tag="ring", name="op")
                nc.tensor.matmul(op[:], lhsT=wo_sb[:], rhs=ynorm[:], start=True, stop=True)
                out_sb = epool.tile([128, I_CHUNK], F32, tag="out_sb")
                nc.scalar.activation(
                    out_sb[:], op[:], mybir.ActivationFunctionType.Copy
                )
                nc.sync.dma_start(out[:, isl], out_sb[:])

            def emit_window(slots, w, nact, last):
                # slots: sim slots first, then proj slots (exp reads a
                # contiguous sim prefix; proj regions feed their ACT copy)
                sims = [s for s in slots if s[0] == "sim"]
                width = 512 * len(slots)
                simw = pring.tile([128, width], F32, tag="ring", name="simw")
                for k, (_, I, J, h) in enumerate(sims):
                    nc.tensor.matmul(
                        simw[:, 512 * k : 512 * (k + 1)],
                        lhsT=k_all[32 * h : 32 * h + 32, 128 * J : 128 * (J + 1)],
                        rhs=q_all[32 * h : 32 * h + 32, I_CHUNK * I : I_CHUNK * (I + 1)],
                        start=True,
                        stop=True,
                        tile_position=(32 * h, 0),
                    )
                # proj writes BEFORE the exp read: tile-granularity dep
                # tracking must never chain a PE write behind the exp
                for k, (_, kind, g) in enumerate(slots[len(sims) :]):
                    sl = slice(512 * (len(sims) + k), 512 * (len(sims) + k + 1))
                    emit_proj_unit(kind, g, simw, sl)
                ew = 512 * len(sims)
                pTw = ptpool.tile([128, ew], BF16, tag="pT", name="pTw")
                if nact:
                    nc.scalar.activation(
                        pTw[:], simw[:, 0:ew], mybir.ActivationFunctionType.Exp
                    )
                else:
                    nc.vector.tensor_scalar(
                        pTw[:].bitcast(I16),
                        simw[:, 0:ew],
                        A_EXP,
                        B_EXP,
                        mybir.AluOpType.mult,
                        mybir.AluOpType.add,
                    )
                flush(w)
                # PV defers TWO bodies so the exp latency chain (sims -> sem
                # -> exp -> sem -> PV) never stalls the PE: micro-idles would
                # re-throttle the HAM clock gate to 1.2GHz and double every
                # matmul's duration. (+1 at the stream tail to shorten drain.)
                d = 1 if last else 2
                schedule(w + d, lambda: emit_pv(sims, pTw, w + d))

            # slot stream: per i-chunk 128 head-sims; proj units ride in
            # i-chunk 0's stream (one per 8 sims — each unit's ~0.5-1us PE
            # burst must fit the per-window slack or it trips a stall that
            # HAM-cools the PE), so they never steal a PSUM ring slot.
            # v0/k1 go into the first windows; k0/q0 prime via the prologue.
            proj_units = [
                ("v", 1), ("k", 2), ("v", 2), ("k", 3), ("v", 3), ("k", 4),
                ("v", 4), ("k", 5), ("v", 5), ("k", 6), ("v", 6), ("k", 7),
                ("v", 7), ("q", 1), ("q", 2), ("q", 3),
            ]
            stream = []
            uq = list(proj_units)
            for I in range(N_I):
                for t in range(N_J * HEADS):
                    stream.append(("sim", I, t // 4, t % 4))
                    if I == 0 and t == 1:
                        stream.append(("proj", "v", 0))
                    elif I == 0 and t == 3:
                        stream.append(("proj", "k", 1))
                    elif I == 0 and uq and t % 8 == 7:
                        stream.append(("proj",) + uq.pop(0))
            assert not uq

            # pack 3 slots per window, sims first within a window; force ACT
            # windows right after each i-chunk boundary (the DVE is busy with
            # the previous chunk's epilogue there)
            windows = []
            for w0 in range(0, len(stream), 3):
                slots = stream[w0 : w0 + 3]
                slots.sort(key=lambda s: s[0] != "sim")
                windows.append(slots)

            emit_proj_batch([("k", 0), ("q", 0)])
            for w, slots in enumerate(windows):
                if w == 1:
                    emit_ones_memset()
                sims = [s for s in slots if s[0] == "sim"]
                nact = ENGINE_PATTERN[w % len(ENGINE_PATTERN)] == "A"
                if sims and sims[0][2] <= 3 and sims[0][1] > 0:
                    nact = True  # chunk-boundary: epilogue owns the DVE
                if w >= len(windows) - 3:
                    nact = True  # tail: final epilogue owns the DVE
                emit_window(slots, w, nact, last=(w >= len(windows) - 2))
            w = len(windows)
            while pending:
                flush(w)
                w += 1

    nc.compile()
    return nc


def kernel(x, w_qkv, w_out, b_out, _trace=False):
    if "nc" not in _NC_CACHE:
        _NC_CACHE["nc"] = _build_nc()
    nc = _NC_CACHE["nc"]

    x = np.asarray(x, dtype=np.float32).reshape(B, C, N)
    w_qkv = np.asarray(w_qkv, dtype=np.float32)
    w_out = np.asarray(w_out, dtype=np.float32)
    b_out = np.asarray(b_out, dtype=np.float32)

    # SCALE folded into wq so sims arrive pre-scaled (both exp paths use raw)
    wq = np.ascontiguousarray(w_qkv[0:C].T * SCALE).astype(NPBF16)
    wk = np.ascontiguousarray(w_qkv[C : 2 * C].T).astype(NPBF16)
    wv = np.ascontiguousarray(w_qkv[2 * C : 3 * C].T).astype(NPBF16)
    wo = np.ascontiguousarray(w_out.T).astype(NPBF16)

    in_maps = []
    for core in range(8):
        b, half = core >> 1, core & 1
        xb = x[b]
        if half:
            xb = np.concatenate([xb[:, NQ:], xb[:, :NQ]], axis=1)
        xb_c = np.ascontiguousarray(
            xb.reshape(C, N // 512, 512).transpose(1, 0, 2)
        ).astype(NPBF16)
        in_maps.append(
            {
                "x": xb_c,
                "wq": wq,
                "wk": wk,
                "wv": wv,
                "wo": wo,
            }
        )

    res = run_bass_kernel_spmd(nc, in_maps, list(range(8)), trace=_trace)

    full = np.empty((B, C, N), np.float32)
    for core in range(8):
        b, half = core >> 1, core & 1
        full[b][:, NQ * half : NQ * (half + 1)] = res.results[core]["out"]
    full += b_out[None, :, None]
    out = full.reshape(B, C, 64, 64)
    if _trace:
        return out, res
    return out


# revision 36
# speedup vs baseline: 1.0307x; 1.0004x over previous
"""TRN2 Bass kernel for nn_Attention_17935783428543.

Reference computation (per batch b of 4):
  qkv = w_qkv @ X        (X = x[b] as [C=128, N=4096])
  per head h (4 heads, d=32): sim = (q_h * scale)^T k_h ; P = softmax(sim)
  y_h = P @ v_h^T ; out = w_out @ concat_h(y_h^T) + b_out

Sharding: 8 cores = 4 batches x 2 query-halves. Each core computes the full
attention for its batch restricted to 2048 query pixels (all 4096 keys), all
4 heads, including QKV projection and the output projection. No collectives.
The query-half assignment uses a host-side rotation of x's pixel axis so all
8 cores run the identical SPMD graph: queries are always columns 0:2048.

Per-core design (all matmuls bf16, PSUM f32). The kernel's wall is the
softmax exp: 33.5M elements/core, and ACT alone runs 1 elem/lane/cycle
@1.2GHz (~262us with overheads). Two levers vs the previous version:

  1. exp is SPLIT between ACT and DVE. DVE windows compute exp with a
     Schraudolph bit-trick in ONE tensor_scalar: i16 = int(sim*a + b) where
     a = 2^7/ln2, b = 127*2^7 - c. The int16 bit pattern IS the bf16 of
     exp(sim) (rel err ~1.8% rms, softmax cancels the constant factor;
     measured end-to-end rel err 5.9e-3 vs the 2e-2 gate). Window pattern
     A,D,A,D,A balances ACT (1573ns/1536-window) vs DVE (1795ns + its
     epilogue work); chunk-boundary windows force ACT so the DVE can run
     the epilogue.
  2. softmax denominators are FREE: the P@V matmuls use lhsT = [v_h | ones32]
     (M=64; matmul cost depends only on N) so each PV matmul accumulates
     y_h^T in rows 0:32 and the broadcast row-sums in rows 32:64 of its
     64-row block. The previous pair-sum DVE ops + ones-matmul stream are
     gone entirely.

Layouts: sim^T tiles [j=128, i=512] per head via K=32 row-packed
tile_position (heads on PE row groups 32h); three head-sims per [128,1536]
PSUM window (3 banks), ring double-buffered (2x3 banks). PV col-packs heads
pairwise: acc tile t = h>>1 holds [y_h|sums_h, y_h'|sums_h'] (2 banks).
Epilogue per i-chunk (split over three window bodies to keep DVE bursts off
the exp stream): whole-tile fast reciprocal, 4x [32,512] ynorm mults, w_out^T
projection, ACT-copy to SBUF, DMA out. The outproj PSUM rides the sim ring
(borrowing an acc bank would chain it behind the NEXT chunk's accumulation
via the pool's generation order); b_out is added on the host. QKV projection
units ride inside window slots (one per 8 sims) with ACT-side copies, so
they never steal a PSUM ring slot; their PE bursts are the main residual
stall source. PV emission is deferred two bodies: the exp latency chain
(sims -> sem -> exp -> sem -> PV) must never idle the PE, because HAM
re-throttles an idling PE to 1.2GHz and doubles every matmul duration.
Measured: ~250us (baseline 333us): ACT ~160us, DVE ~155us, PE ~185us busy.
"""

import math

import numpy as np
import ml_dtypes

import concourse.mybir as mybir
import concourse.tile as tile
from concourse import bacc
from concourse.bass_utils import run_bass_kernel_spmd

F32 = mybir.dt.float32
BF16 = mybir.dt.bfloat16
I16 = mybir.dt.int16
NPBF16 = ml_dtypes.bfloat16

B = 4
C = 128
HEADS = 4
D = 32
N = 4096          # pixels per batch (64*64)
NQ = 2048         # query pixels per core
SCALE = D ** -0.5
I_CHUNK = 512
J_CHUNK = 128
N_I = NQ // I_CHUNK     # 4
N_J = N // J_CHUNK      # 32

# Schraudolph exp in bf16-bit domain (sims arrive pre-scaled via wq):
#   bf16_bits(exp(s)) ~= int16(s * 2^7/ln2 + (127*2^7 - c))
# c=3.25 splits the optimum between round-to-nearest (3.5) and truncating
# (3.0) f32->i16 conversion; softmax cancels the resulting constant factor.
A_EXP = 128.0 / math.log(2.0)
B_EXP = 16256.0 - 3.25

# per-window exp engine: A=ACT exact exp, D=DVE Schraudolph (3:2 balance)
ENGINE_PATTERN = "ADADA"

_NC_CACHE = {}


def _build_nc():
    nc = bacc.Bacc("TRN2", target_bir_lowering=False, debug=False, num_devices=8)

    x = nc.dram_tensor("x", [N // 512, C, 512], BF16, kind="ExternalInput").ap()
    wq = nc.dram_tensor("wq", [C, C], BF16, kind="ExternalInput").ap()
    wk = nc.dram_tensor("wk", [C, C], BF16, kind="ExternalInput").ap()
    wv = nc.dram_tensor("wv", [C, C], BF16, kind="ExternalInput").ap()
    wo = nc.dram_tensor("wo", [C, C], BF16, kind="ExternalInput").ap()
    out = nc.dram_tensor("out", [C, NQ], F32, kind="ExternalOutput").ap()

    with tile.TileContext(nc) as tc:
        with (
            tc.tile_pool(name="const", bufs=1) as cpool,
            tc.tile_pool(name="acts", bufs=1) as apool,
            tc.tile_pool(name="pt", bufs=6) as ptpool,
            tc.tile_pool(name="epi", bufs=2) as epool,
            tc.tile_pool(name="psum_ring", bufs=2, space="PSUM") as pring,
            tc.tile_pool(name="psum_acc", bufs=1, space="PSUM") as pacc,
        ):
            # ---- constants / weights ----
            wq_sb = cpool.tile([C, C], BF16, tag="wq")
            nc.sync.dma_start(wq_sb[:], wq)
            wk_sb = cpool.tile([C, C], BF16, tag="wk")
            nc.sync.dma_start(wk_sb[:], wk)
            wv_sb = cpool.tile([C, C], BF16, tag="wv")
            nc.sync.dma_start(wv_sb[:], wv)
            wo_sb = cpool.tile([C, C], BF16, tag="wo")
            nc.sync.dma_start(wo_sb[:], wo)

            # warm the ACT exp table during the DMA prologue
            warm = cpool.tile([1, 1], F32, tag="warm")
            nc.vector.memset(warm[:], 0.0)
            nc.scalar.activation(warm[:], warm[:], mybir.ActivationFunctionType.Exp)

            # ---- x DMA (8 chunks so early projections can start early) ----
            x_sb = apool.tile([C, N], BF16, tag="x")
            for g in range(N // 512):
                nc.gpsimd.dma_start(x_sb[:, 512 * g : 512 * (g + 1)], x[g])

            q_all = apool.tile([C, NQ], BF16, tag="q")    # [4h*32c', i]
            k_all = apool.tile([C, N], BF16, tag="k")     # [4h*32c', j]
            # v^T with interleaved ones columns: per chunk J the 256-col block
            # is [v_h0(32) | 1(32) | v_h1 | 1 | v_h2 | 1 | v_h3 | 1], rows = j
            vT_all = apool.tile([C, 256 * N_J], BF16, tag="vT")

            def emit_ones_memset():
                # fill only the 128 ones-slots (odd 32-col slots), strided
                ones_ap = vT_all[:].rearrange("p (a d) -> p a d", d=64)[:, :, 32:64]
                nc.vector.memset(ones_ap, 1.0)

            # ---- projection units ----
            # A unit occupies one 512-col region of a PSUM tile (one bank):
            # either a standalone prologue ring tile or a window slot.
            def emit_proj_unit(kind, g, ps, sl, dve_copy=False):
                if kind == "q" and dve_copy:
                    nc.tensor.matmul(
                        ps[:, sl],
                        lhsT=wq_sb[:],
                        rhs=x_sb[:, 512 * g : 512 * (g + 1)],
                        start=True,
                        stop=True,
                    )
                    nc.vector.tensor_copy(q_all[:, 512 * g : 512 * (g + 1)], ps[:, sl])
                elif kind == "q":
                    nc.tensor.matmul(
                        ps[:, sl],
                        lhsT=wq_sb[:],
                        rhs=x_sb[:, 512 * g : 512 * (g + 1)],
                        start=True,
                        stop=True,
                    )
                    nc.scalar.activation(
                        q_all[:, 512 * g : 512 * (g + 1)], ps[:, sl],
                        mybir.ActivationFunctionType.Copy,
                    )
                elif kind == "k":
                    nc.tensor.matmul(
                        ps[:, sl],
                        lhsT=wk_sb[:],
                        rhs=x_sb[:, 512 * g : 512 * (g + 1)],
                        start=True,
                        stop=True,
                    )
                    nc.scalar.activation(
                        k_all[:, 512 * g : 512 * (g + 1)], ps[:, sl],
                        mybir.ActivationFunctionType.Copy,
                    )
                else:  # v^T: per 128-pixel chunk, out[j, o'] = x^T wv
                    for c4 in range(4):
                        J = 4 * g + c4
                        nc.tensor.matmul(
                            ps[:, sl.start + 128 * c4 : sl.start + 128 * (c4 + 1)],
                            lhsT=x_sb[:, 128 * J : 128 * (J + 1)],
                            rhs=wv_sb[:],
                            start=True,
                            stop=True,
                        )
                    # scatter the 16 [?,32] head-slices into the v-slots
                    # (even 32-col slots of the 4 chunks' 256-col blocks)
                    dst = (
                        vT_all[:, 1024 * g : 1024 * (g + 1)]
                        .rearrange("p (a d) -> p a d", d=64)[:, :, 0:32]
                    )
                    src = ps[:, sl].rearrange("p (a d) -> p a d", d=32)
                    nc.scalar.activation(dst, src, mybir.ActivationFunctionType.Copy)

            def emit_proj_batch(units):
                width = 512 * len(units)
                ps = pring.tile([128, width], F32, tag="ring", name="proj_ps")
                for u, (kind, g) in enumerate(units):
                    emit_proj_unit(
                        kind, g, ps, slice(512 * u, 512 * (u + 1)), dve_copy=True
                    )

            # ---- attention stream ----
            # deferral scheduler: actions run at the start of their due body,
            # in insertion order (bodies past the window list keep flushing)
            acc_tiles = {}
            pending = []  # (due_body, fn)

            def schedule(due, fn):
                pending.append((due, fn))

            def flush(w):
                i = 0
                while i < len(pending):
                    due, fn = pending[i]
                    if due <= w:
                        pending.pop(i)
                        fn()
                    else:
                        i += 1

            def emit_pv_slot(I, J, h, pTw, k, w):
                if I not in acc_tiles:
                    acc_tiles[I] = (
                        pacc.tile([128, I_CHUNK], F32, tag="acc0", name="acc0"),
                        pacc.tile([128, I_CHUNK], F32, tag="acc1", name="acc1"),
                    )
                base = 64 * (h & 1)
                nc.tensor.matmul(
                    acc_tiles[I][h >> 1][base : base + 64, :],
                    lhsT=vT_all[:, 256 * J + 64 * h : 256 * J + 64 * h + 64],
                    rhs=pTw[:, 512 * k : 512 * (k + 1)],
                    start=(J == 0),
                    stop=(J == N_J - 1),
                    tile_position=(0, base),
                )
                if (J, h) == (N_J - 1, HEADS - 1):
                    st = emit_epilogue1a(I)
                    schedule(w + 1, lambda st=st: emit_epilogue1b(st))
                    schedule(w + 2, lambda I=I, st=st: emit_epilogue2(I, st))

            def emit_pv(slots, pTw, w):
                for k, (_, I, J, h) in enumerate(slots):
                    emit_pv_slot(I, J, h, pTw, k, w)

            def emit_epilogue1a(I):
                acc0, acc1 = acc_tiles.pop(I)
                r0 = epool.tile([128, I_CHUNK], F32, tag="r0")
                nc.vector.reciprocal_approx_fast(r0[:], acc0[:])
                r1 = epool.tile([128, I_CHUNK], F32, tag="r1")
                nc.vector.reciprocal_approx_fast(r1[:], acc1[:])
                ynorm = epool.tile([128, I_CHUNK], BF16, tag="ynorm")
                for h in (0, 1):
                    nc.vector.tensor_tensor(
                        ynorm[32 * h : 32 * h + 32, :],
                        acc0[64 * h : 64 * h + 32, :],
                        r0[64 * h + 32 : 64 * h + 64, :],
                        mybir.AluOpType.mult,
                    )
                return (acc1, r1, ynorm)

            def emit_epilogue1b(st):
                acc1, r1, ynorm = st
                for h in (2, 3):
                    b = 64 * (h & 1)
                    nc.vector.tensor_tensor(
                        ynorm[32 * h : 32 * h + 32, :],
                        acc1[b : b + 32, :],
                        r1[b + 32 : b + 64, :],
                        mybir.AluOpType.mult,
                    )

            def emit_epilogue2(I, st):
                ynorm = st[2]
                isl = slice(I_CHUNK * I, I_CHUNK * (I + 1))
                # outproj PSUM rides the ring (one window slot per i-chunk):
                # borrowing an acc bank makes the pool's generation order
                # chain it behind the NEXT chunk's accumulation
                op = pring.tile([128, I_CHUNK], F32, tag="ring", name="op")
                nc.tensor.matmul(op[:], lhsT=wo_sb[:], rhs=ynorm[:], start=True, stop=True)
                out_sb = epool.tile([128, I_CHUNK], F32, tag="out_sb")
                nc.scalar.activation(
                    out_sb[:], op[:], mybir.ActivationFunctionType.Copy
                )
                nc.sync.dma_start(out[:, isl], out_sb[:])

            def emit_window(slots, w, nact, last):
                # slots: sim slots first, then proj slots (exp reads a
                # contiguous sim prefix; proj regions feed their ACT copy)
                sims = [s for s in slots if s[0] == "sim"]
                width = 512 * len(slots)
                simw = pring.tile([128, width], F32, tag="ring", name="simw")
                for k, (_, I, J, h) in enumerate(sims):
                    nc.tensor.matmul(
                        simw[:, 512 * k : 512 * (k + 1)],
                        lhsT=k_all[32 * h : 32 * h + 32, 128 * J : 128 * (J + 1)],
                        rhs=q_all[32 * h : 32 * h + 32, I_CHUNK * I : I_CHUNK * (I + 1)],
                        start=True,
                        stop=True,
                        tile_position=(32 * h, 0),
                    )
                # proj writes BEFORE the exp read: tile-granularity dep
                # tracking must never chain a PE write behind the exp
                for k, (_, kind, g) in enumerate(slots[len(sims) :]):
                    sl = slice(512 * (len(sims) + k), 512 * (len(sims) + k + 1))
                    emit_proj_unit(kind, g, simw, sl)
                ew = 512 * len(sims)
                pTw = ptpool.tile([128, ew], BF16, tag="pT", name="pTw")
                if nact:
                    nc.scalar.activation(
                        pTw[:], simw[:, 0:ew], mybir.ActivationFunctionType.Exp
                    )
                else:
                    nc.vector.tensor_scalar(
                        pTw[:].bitcast(I16),
                        simw[:, 0:ew],
                        A_EXP,
                        B_EXP,
                        mybir.AluOpType.mult,
                        mybir.AluOpType.add,
                    )
                flush(w)
                # PV defers TWO bodies so the exp latency chain (sims -> sem
                # -> exp -> sem -> PV) never stalls the PE: micro-idles would
                # re-throttle the HAM clock gate to 1.2GHz and double every
                # matmul's duration. (+1 at the stream tail to shorten drain.)
                d = 1 if last else 2
                schedule(w + d, lambda: emit_pv(sims, pTw, w + d))

            # slot stream: per i-chunk 128 head-sims; proj units ride in
            # i-chunk 0's stream (one per 8 sims — each unit's ~0.5-1us PE
            # burst must fit the per-window slack or it trips a stall that
            # HAM-cools the PE), so they never steal a PSUM ring slot.
            # v0/k1 go into the first windows; k0/q0 prime via the prologue.
            proj_units = [
                ("v", 1), ("k", 2), ("v", 2), ("k", 3), ("v", 3), ("k", 4),
                ("v", 4), ("k", 5), ("v", 5), ("k", 6), ("v", 6), ("k", 7),
                ("v", 7), ("q", 1), ("q", 2), ("q", 3),
            ]
            stream = []
            uq = list(proj_units)
            for I in range(N_I):
                for t in range(N_J * HEADS):
                    stream.append(("sim", I, t // 4, t % 4))
                    if I == 0 and t == 1:
                        stream.append(("proj", "v", 0))
                    elif I == 0 and t == 3:
                        stream.append(("proj", "k", 1))
                    elif I == 0 and uq and t % 8 == 7:
                        stream.append(("proj",) + uq.pop(0))
            assert not uq

            # pack 3 slots per window, sims first within a window; force ACT
            # windows right after each i-chunk boundary (the DVE is busy with
            # the previous chunk's epilogue there)
            windows = []
            for w0 in range(0, len(stream), 3):
                slots = stream[w0 : w0 + 3]
                slots.sort(key=lambda s: s[0] != "sim")
                windows.append(slots)

            emit_proj_batch([("k", 0), ("q", 0)])
            for w, slots in enumerate(windows):
                if w == 1:
                    emit_ones_memset()
                sims = [s for s in slots if s[0] == "sim"]
                nact = ENGINE_PATTERN[w % len(ENGINE_PATTERN)] == "A"
                if sims and sims[0][2] <= 3 and sims[0][1] > 0:
                    nact = True  # chunk-boundary: epilogue owns the DVE
                if w >= len(windows) - 3:
                    nact = True  # tail: final epilogue owns the DVE
                emit_window(slots, w, nact, last=(w >= len(windows) - 2))
            w = len(windows)
            while pending:
                flush(w)
                w += 1

    nc.compile()
    return nc


def kernel(x, w_qkv, w_out, b_out, _trace=False):
    if "nc" not in _NC_CACHE:
        _NC_CACHE["nc"] = _build_nc()
    nc = _NC_CACHE["nc"]

    x = np.asarray(x, dtype=np.float32).reshape(B, C, N)
    w_qkv = np.asarray(w_qkv, dtype=np.float32)
    w_out = np.asarray(w_out, dtype=np.float32)
    b_out = np.asarray(b_out, dtype=np.float32)

    # SCALE folded into wq so sims arrive pre-scaled (both exp paths use raw)
    wq = np.ascontiguousarray(w_qkv[0:C].T * SCALE).astype(NPBF16)
    wk = np.ascontiguousarray(w_qkv[C : 2 * C].T).astype(NPBF16)
    wv = np.ascontiguousarray(w_qkv[2 * C : 3 * C].T).astype(NPBF16)
    wo = np.ascontiguousarray(w_out.T).astype(NPBF16)

    in_maps = []
    for core in range(8):
        b, half = core >> 1, core & 1
        xb = x[b]
        if half:
            xb = np.concatenate([xb[:, NQ:], xb[:, :NQ]], axis=1)
        xb_c = np.ascontiguousarray(
            xb.reshape(C, N // 512, 512).transpose(1, 0, 2)
        ).astype(NPBF16)
        in_maps.append(
            {
                "x": xb_c,
                "wq": wq,
                "wk": wk,
                "wv": wv,
                "wo": wo,
            }
        )

    res = run_bass_kernel_spmd(nc, in_maps, list(range(8)), trace=_trace)

    full = np.empty((B, C, N), np.float32)
    for core in range(8):
        b, half = core >> 1, core & 1
        full[b][:, NQ * half : NQ * (half + 1)] = res.results[core]["out"]
    full += b_out[None, :, None]
    out = full.reshape(B, C, 64, 64)
    if _trace:
        return out, res
    return out


# revision 37
# speedup vs baseline: 1.0337x; 1.0029x over previous
"""TRN2 Bass kernel for nn_Attention_17935783428543.

Reference computation (per batch b of 4):
  qkv = w_qkv @ X        (X = x[b] as [C=128, N=4096])
  per head h (4 heads, d=32): sim = (q_h * scale)^T k_h ; P = softmax(sim)
  y_h = P @ v_h^T ; out = w_out @ concat_h(y_h^T) + b_out

Sharding: 8 cores = 4 batches x 2 query-halves. Each core computes the full
attention for its batch restricted to 2048 query pixels (all 4096 keys), all
4 heads, including QKV projection and the output projection. No collectives.
The query-half assignment uses a host-side rotation of x's pixel axis so all
8 cores run the identical SPMD graph: queries are always columns 0:2048.

Per-core design (all matmuls bf16, PSUM f32). The kernel's wall is the
softmax exp: 33.5M elements/core, and ACT alone runs 1 elem/lane/cycle
@1.2GHz (~262us with overheads). Two levers vs the previous version:

  1. exp is SPLIT between ACT and DVE. DVE windows compute exp with a
     Schraudolph bit-trick in ONE tensor_scalar: i16 = int(sim*a + b) where
     a = 2^7/ln2, b = 127*2^7 - c. The int16 bit pattern IS the bf16 of
     exp(sim) (rel err ~1.8% rms, softmax cancels the constant factor;
     measured end-to-end rel err 5.9e-3 vs the 2e-2 gate). Window pattern
     A,D,A,D,A balances ACT (1573ns/1536-window) vs DVE (1795ns + its
     epilogue work); chunk-boundary windows force ACT so the DVE can run
     the epilogue.
  2. softmax denominators are FREE: the P@V matmuls use lhsT = [v_h | ones32]
     (M=64; matmul cost depends only on N) so each PV matmul accumulates
     y_h^T in rows 0:32 and the broadcast row-sums in rows 32:64 of its
     64-row block. The previous pair-sum DVE ops + ones-matmul stream are
     gone entirely.

Layouts: sim^T tiles [j=128, i=512] per head via K=32 row-packed
tile_position (heads on PE row groups 32h); three head-sims per [128,1536]
PSUM window (3 banks), ring double-buffered (2x3 banks). PV col-packs heads
pairwise: acc tile t = h>>1 holds [y_h|sums_h, y_h'|sums_h'] (2 banks).
Epilogue per i-chunk (split over three window bodies to keep DVE bursts off
the exp stream): whole-tile fast reciprocal, 4x [32,512] ynorm mults, w_out^T
projection, ACT-copy to SBUF, DMA out. The outproj PSUM rides the sim ring
(borrowing an acc bank would chain it behind the NEXT chunk's accumulation
via the pool's generation order); b_out is added on the host. QKV projection
units ride inside window slots (one per 8 sims) with ACT-side copies, so
they never steal a PSUM ring slot; their PE bursts are the main residual
stall source. PV emission is deferred two bodies: the exp latency chain
(sims -> sem -> exp -> sem -> PV) must never idle the PE, because HAM
re-throttles an idling PE to 1.2GHz and doubles every matmul duration.
Measured: ~250us (baseline 333us): ACT ~160us, DVE ~155us, PE ~185us busy.
"""

import math

import numpy as np
import ml_dtypes

import concourse.mybir as mybir
import concourse.tile as tile
from concourse import bacc
from concourse.bass_utils import run_bass_kernel_spmd

F32 = mybir.dt.float32
BF16 = mybir.dt.bfloat16
I16 = mybir.dt.int16
NPBF16 = ml_dtypes.bfloat16

B = 4
C = 128
HEADS = 4
D = 32
N = 4096          # pixels per batch (64*64)
NQ = 2048         # query pixels per core
SCALE = D ** -0.5
I_CHUNK = 512
J_CHUNK = 128
N_I = NQ // I_CHUNK     # 4
N_J = N // J_CHUNK      # 32

# Schraudolph exp in bf16-bit domain (sims arrive pre-scaled via wq):
#   bf16_bits(exp(s)) ~= int16(s * 2^7/ln2 + (127*2^7 - c))
# c=3.25 splits the optimum between round-to-nearest (3.5) and truncating
# (3.0) f32->i16 conversion; softmax cancels the resulting constant factor.
A_EXP = 128.0 / math.log(2.0)
B_EXP = 16256.0 - 3.25

# per-window exp engine: A=ACT exact exp, D=DVE Schraudolph (3:2 balance)
ENGINE_PATTERN = "ADADA"

_NC_CACHE = {}


def _build_nc():
    nc = bacc.Bacc("TRN2", target_bir_lowering=False, debug=False, num_devices=8)

    x = nc.dram_tensor("x", [N // 512, C, 512], BF16, kind="ExternalInput").ap()
    wq = nc.dram_tensor("wq", [C, C], BF16, kind="ExternalInput").ap()
    wk = nc.dram_tensor("wk", [C, C], BF16, kind="ExternalInput").ap()
    wv = nc.dram_tensor("wv", [C, C], BF16, kind="ExternalInput").ap()
    wo = nc.dram_tensor("wo", [C, C], BF16, kind="ExternalInput").ap()
    out = nc.dram_tensor("out", [C, NQ], F32, kind="ExternalOutput").ap()

    with tile.TileContext(nc) as tc:
        with (
            tc.tile_pool(name="const", bufs=1) as cpool,
            tc.tile_pool(name="acts", bufs=1) as apool,
            tc.tile_pool(name="pt", bufs=6) as ptpool,
            tc.tile_pool(name="epi", bufs=2) as epool,
            tc.tile_pool(name="psum_ring", bufs=2, space="PSUM") as pring,
            tc.tile_pool(name="psum_acc", bufs=1, space="PSUM") as pacc,
        ):
            # ---- constants / weights ----
            wq_sb = cpool.tile([C, C], BF16, tag="wq")
            nc.sync.dma_start(wq_sb[:], wq)
            wk_sb = cpool.tile([C, C], BF16, tag="wk")
            nc.sync.dma_start(wk_sb[:], wk)
            wv_sb = cpool.tile([C, C], BF16, tag="wv")
            nc.sync.dma_start(wv_sb[:], wv)
            wo_sb = cpool.tile([C, C], BF16, tag="wo")
            nc.sync.dma_start(wo_sb[:], wo)

            # warm the ACT exp table during the DMA prologue
            warm = cpool.tile([1, 1], F32, tag="warm")
            nc.vector.memset(warm[:], 0.0)
            nc.scalar.activation(warm[:], warm[:], mybir.ActivationFunctionType.Exp)

            # warm the PE's HAM clock gate during the DMA wait: ~3.5us of
            # dummy matmuls on zeroed scratch SBUF flips the gate to 8/8
            # before the first real matmul, so the prologue projections and
            # the first windows run at 2.4GHz instead of 1.2GHz. The dummy
            # PSUM tile is start=True-overwritten garbage, never read, and
            # the dummies finish before the first x chunk lands.
            scratch = cpool.tile([128, 512], BF16, tag="pewarm_in")
            nc.vector.memset(scratch[:], 0.0)
            pe_warm = pring.tile([128, 512], F32, tag="ring", name="pe_warm")
            for _ in range(8):
                nc.tensor.matmul(
                    pe_warm[:],
                    lhsT=scratch[:, 0:128],
                    rhs=scratch[:],
                    start=True,
                    stop=True,
                )

            # ---- x DMA (8 chunks so early projections can start early) ----
            x_sb = apool.tile([C, N], BF16, tag="x")
            for g in range(N // 512):
                nc.gpsimd.dma_start(x_sb[:, 512 * g : 512 * (g + 1)], x[g])

            q_all = apool.tile([C, NQ], BF16, tag="q")    # [4h*32c', i]
            k_all = apool.tile([C, N], BF16, tag="k")     # [4h*32c', j]
            # v^T with interleaved ones columns: per chunk J the 256-col block
            # is [v_h0(32) | 1(32) | v_h1 | 1 | v_h2 | 1 | v_h3 | 1], rows = j
            vT_all = apool.tile([C, 256 * N_J], BF16, tag="vT")

            def emit_ones_memset():
                # fill only the 128 ones-slots (odd 32-col slots), strided
                ones_ap = vT_all[:].rearrange("p (a d) -> p a d", d=64)[:, :, 32:64]
                nc.vector.memset(ones_ap, 1.0)

            # ---- projection units ----
            # A unit occupies one 512-col region of a PSUM tile (one bank):
            # either a standalone prologue ring tile or a window slot.
            def emit_proj_unit(kind, g, ps, sl, dve_copy=False):
                if kind == "q" and dve_copy:
                    nc.tensor.matmul(
                        ps[:, sl],
                        lhsT=wq_sb[:],
                        rhs=x_sb[:, 512 * g : 512 * (g + 1)],
                        start=True,
                        stop=True,
                    )
                    nc.vector.tensor_copy(q_all[:, 512 * g : 512 * (g + 1)], ps[:, sl])
                elif kind == "q":
                    nc.tensor.matmul(
                        ps[:, sl],
                        lhsT=wq_sb[:],
                        rhs=x_sb[:, 512 * g : 512 * (g + 1)],
                        start=True,
                        stop=True,
                    )
                    nc.scalar.activation(
                        q_all[:, 512 * g : 512 * (g + 1)], ps[:, sl],
                        mybir.ActivationFunctionType.Copy,
                    )
                elif kind == "k":
                    nc.tensor.matmul(
                        ps[:, sl],
                        lhsT=wk_sb[:],
                        rhs=x_sb[:, 512 * g : 512 * (g + 1)],
                        start=True,
                        stop=True,
                    )
                    nc.scalar.activation(
                        k_all[:, 512 * g : 512 * (g + 1)], ps[:, sl],
                        mybir.ActivationFunctionType.Copy,
                    )
                else:  # v^T: per 128-pixel chunk, out[j, o'] = x^T wv
                    for c4 in range(4):
                        J = 4 * g + c4
                        nc.tensor.matmul(
                            ps[:, sl.start + 128 * c4 : sl.start + 128 * (c4 + 1)],
                            lhsT=x_sb[:, 128 * J : 128 * (J + 1)],
                            rhs=wv_sb[:],
                            start=True,
                            stop=True,
                        )
                    # scatter the 16 [?,32] head-slices into the v-slots
                    # (even 32-col slots of the 4 chunks' 256-col blocks)
                    dst = (
                        vT_all[:, 1024 * g : 1024 * (g + 1)]
                        .rearrange("p (a d) -> p a d", d=64)[:, :, 0:32]
                    )
                    src = ps[:, sl].rearrange("p (a d) -> p a d", d=32)
                    nc.scalar.activation(dst, src, mybir.ActivationFunctionType.Copy)

            def emit_proj_batch(units):
                width = 512 * len(units)
                ps = pring.tile([128, width], F32, tag="ring", name="proj_ps")
                for u, (kind, g) in enumerate(units):
                    emit_proj_unit(
                        kind, g, ps, slice(512 * u, 512 * (u + 1)), dve_copy=True
                    )

            # ---- attention stream ----
            # deferral scheduler: actions run at the start of their due body,
            # in insertion order (bodies past the window list keep flushing)
            acc_tiles = {}
            pending = []  # (due_body, fn)

            def schedule(due, fn):
                pending.append((due, fn))

            def flush(w):
                i = 0
                while i < len(pending):
                    due, fn = pending[i]
                    if due <= w:
                        pending.pop(i)
                        fn()
                    else:
                        i += 1

            def emit_pv_slot(I, J, h, pTw, k, w):
                if I not in acc_tiles:
                    acc_tiles[I] = (
                        pacc.tile([128, I_CHUNK], F32, tag="acc0", name="acc0"),
                        pacc.tile([128, I_CHUNK], F32, tag="acc1", name="acc1"),
                    )
                base = 64 * (h & 1)
                nc.tensor.matmul(
                    acc_tiles[I][h >> 1][base : base + 64, :],
                    lhsT=vT_all[:, 256 * J + 64 * h : 256 * J + 64 * h + 64],
                    rhs=pTw[:, 512 * k : 512 * (k + 1)],
                    start=(J == 0),
                    stop=(J == N_J - 1),
                    tile_position=(0, base),
                )
                if (J, h) == (N_J - 1, HEADS - 1):
                    st = emit_epilogue1a(I)
                    schedule(w + 1, lambda st=st: emit_epilogue1b(st))
                    schedule(w + 2, lambda I=I, st=st: emit_epilogue2(I, st))

            def emit_pv(slots, pTw, w):
                for k, (_, I, J, h) in enumerate(slots):
                    emit_pv_slot(I, J, h, pTw, k, w)

            def emit_epilogue1a(I):
                acc0, acc1 = acc_tiles.pop(I)
                r0 = epool.tile([128, I_CHUNK], F32, tag="r0")
                nc.vector.reciprocal_approx_fast(r0[:], acc0[:])
                r1 = epool.tile([128, I_CHUNK], F32, tag="r1")
                nc.vector.reciprocal_approx_fast(r1[:], acc1[:])
                ynorm = epool.tile([128, I_CHUNK], BF16, tag="ynorm")
                for h in (0, 1):
                    nc.vector.tensor_tensor(
                        ynorm[32 * h : 32 * h + 32, :],
                        acc0[64 * h : 64 * h + 32, :],
                        r0[64 * h + 32 : 64 * h + 64, :],
                        mybir.AluOpType.mult,
                    )
                return (acc1, r1, ynorm)

            def emit_epilogue1b(st):
                acc1, r1, ynorm = st
                for h in (2, 3):
                    b = 64 * (h & 1)
                    nc.vector.tensor_tensor(
                        ynorm[32 * h : 32 * h + 32, :],
                        acc1[b : b + 32, :],
                        r1[b + 32 : b + 64, :],
                        mybir.AluOpType.mult,
                    )

            def emit_epilogue2(I, st):
                ynorm = st[2]
                isl = slice(I_CHUNK * I, I_CHUNK * (I + 1))
                # outproj PSUM rides the ring (one window slot per i-chunk):
                # borrowing an acc bank makes the pool's generation order
                # chain it behind the NEXT chunk's accumulation
                op = pring.tile([128, I_CHUNK], F32, tag="ring", name="op")
                nc.tensor.matmul(op[:], lhsT=wo_sb[:], rhs=ynorm[:], start=True, stop=True)
                out_sb = epool.tile([128, I_CHUNK], F32, tag="out_sb")
                nc.scalar.activation(
                    out_sb[:], op[:], mybir.ActivationFunctionType.Copy
                )
                nc.sync.dma_start(out[:, isl], out_sb[:])

            def emit_window(slots, w, nact, last):
                # slots: sim slots first, then proj slots (exp reads a
                # contiguous sim prefix; proj regions feed their ACT copy)
                sims = [s for s in slots if s[0] == "sim"]
                width = 512 * len(slots)
                simw = pring.tile([128, width], F32, tag="ring", name="simw")
                for k, (_, I, J, h) in enumerate(sims):
                    nc.tensor.matmul(
                        simw[:, 512 * k : 512 * (k + 1)],
                        lhsT=k_all[32 * h : 32 * h + 32, 128 * J : 128 * (J + 1)],
                        rhs=q_all[32 * h : 32 * h + 32, I_CHUNK * I : I_CHUNK * (I + 1)],
                        start=True,
                        stop=True,
                        tile_position=(32 * h, 0),
                    )
                # proj writes BEFORE the exp read: tile-granularity dep
                # tracking must never chain a PE write behind the exp
                for k, (_, kind, g) in enumerate(slots[len(sims) :]):
                    sl = slice(512 * (len(sims) + k), 512 * (len(sims) + k + 1))
                    emit_proj_unit(kind, g, simw, sl)
                ew = 512 * len(sims)
                pTw = ptpool.tile([128, ew], BF16, tag="pT", name="pTw")
                if nact:
                    nc.scalar.activation(
                        pTw[:], simw[:, 0:ew], mybir.ActivationFunctionType.Exp
                    )
                else:
                    nc.vector.tensor_scalar(
                        pTw[:].bitcast(I16),
                        simw[:, 0:ew],
                        A_EXP,
                        B_EXP,
                        mybir.AluOpType.mult,
                        mybir.AluOpType.add,
                    )
                flush(w)
                # PV defers TWO bodies so the exp latency chain (sims -> sem
                # -> exp -> sem -> PV) never stalls the PE: micro-idles would
                # re-throttle the HAM clock gate to 1.2GHz and double every
                # matmul's duration. (+1 at the stream tail to shorten drain.)
                d = 1 if last else 2
                schedule(w + d, lambda: emit_pv(sims, pTw, w + d))

            # slot stream: per i-chunk 128 head-sims; proj units ride in
            # i-chunk 0's stream (one per 8 sims — each unit's ~0.5-1us PE
            # burst must fit the per-window slack or it trips a stall that
            # HAM-cools the PE), so they never steal a PSUM ring slot.
            # v0/k1 go into the first windows; k0/q0 prime via the prologue.
            proj_units = [
                ("v", 1), ("k", 2), ("v", 2), ("k", 3), ("v", 3), ("k", 4),
                ("v", 4), ("k", 5), ("v", 5), ("k", 6), ("v", 6), ("k", 7),
                ("v", 7), ("q", 1), ("q", 2), ("q", 3),
            ]
            stream = []
            uq = list(proj_units)
            for I in range(N_I):
                for t in range(N_J * HEADS):
                    stream.append(("sim", I, t // 4, t % 4))
                    if I == 0 and t == 1:
                        stream.append(("proj", "v", 0))
                    elif I == 0 and t == 3:
                        stream.append(("proj", "k", 1))
                    elif I == 0 and uq and t % 8 == 7:
                        stream.append(("proj",) + uq.pop(0))
            assert not uq

            # pack 3 slots per window, sims first within a window; force ACT
            # windows right after each i-chunk boundary (the DVE is busy with
            # the previous chunk's epilogue there)
            windows = []
            for w0 in range(0, len(stream), 3):
                slots = stream[w0 : w0 + 3]
                slots.sort(key=lambda s: s[0] != "sim")
                windows.append(slots)

            emit_proj_batch([("k", 0), ("q", 0)])
            for w, slots in enumerate(windows):
                if w == 1:
                    emit_ones_memset()
                sims = [s for s in slots if s[0] == "sim"]
                nact = ENGINE_PATTERN[w % len(ENGINE_PATTERN)] == "A"
                if sims and sims[0][2] <= 3 and sims[0][1] > 0:
                    nact = True  # chunk-boundary: epilogue owns the DVE
                if w >= len(windows) - 3:
                    nact = True  # tail: final epilogue owns the DVE
                emit_window(slots, w, nact, last=(w >= len(windows) - 2))
            w = len(windows)
            while pending:
                flush(w)
                w += 1

    nc.compile()
    return nc


def kernel(x, w_qkv, w_out, b_out, _trace=False):
    if "nc" not in _NC_CACHE:
        _NC_CACHE["nc"] = _build_nc()
    nc = _NC_CACHE["nc"]

    x = np.asarray(x, dtype=np.float32).reshape(B, C, N)
    w_qkv = np.asarray(w_qkv, dtype=np.float32)
    w_out = np.asarray(w_out, dtype=np.float32)
    b_out = np.asarray(b_out, dtype=np.float32)

    # SCALE folded into wq so sims arrive pre-scaled (both exp paths use raw)
    wq = np.ascontiguousarray(w_qkv[0:C].T * SCALE).astype(NPBF16)
    wk = np.ascontiguousarray(w_qkv[C : 2 * C].T).astype(NPBF16)
    wv = np.ascontiguousarray(w_qkv[2 * C : 3 * C].T).astype(NPBF16)
    wo = np.ascontiguousarray(w_out.T).astype(NPBF16)

    in_maps = []
    for core in range(8):
        b, half = core >> 1, core & 1
        xb = x[b]
        if half:
            xb = np.concatenate([xb[:, NQ:], xb[:, :NQ]], axis=1)
        xb_c = np.ascontiguousarray(
            xb.reshape(C, N // 512, 512).transpose(1, 0, 2)
        ).astype(NPBF16)
        in_maps.append(
            {
                "x": xb_c,
                "wq": wq,
                "wk": wk,
                "wv": wv,
                "wo": wo,
            }
        )

    res = run_bass_kernel_spmd(nc, in_maps, list(range(8)), trace=_trace)

    full = np.empty((B, C, N), np.float32)
    for core in range(8):
        b, half = core >> 1, core & 1
        full[b][:, NQ * half : NQ * (half + 1)] = res.results[core]["out"]
    full += b_out[None, :, None]
    out = full.reshape(B, C, 64, 64)
    if _trace:
        return out, res
    return out
